# revision 2
# baseline (speedup 1.0000x reference)
"""Trainium2 Bass kernel for nn_CrossChannelAttention.

Reference computation (per batch b, pixel p, with C=128 channels, NUMS=16
groups of HEADS=8 channels, OUT=256):
    fm[g,p]  = relu(sum_h W1[g,h] * x[8g+h, p] + b1[g])          # [16, P]
    feat[(g,d), p] = fm[g,p] * x[d,p]                            # [2048, P]
    out[o,p] = sum_c W2[o,c] * feat[c,p] + b2[o]                 # [256, P]

Strategy: data-parallel over batch B=8 across the 8 NeuronCores (one batch
image per core, params replicated).  Per core:
  - fm via one small matmul (W1 scattered into a [128,16] one-hot-ish lhsT)
  - fm row g broadcast to 128 partitions via a selection matmul
    (lhsT = one-hot column block of sel [16,128], rhs = fm [16,512])
  - feat_g = x * fm_rep_g on the vector engine (PSUM x SBUF -> SBUF)
  - out accumulated over the 16 groups in PSUM:  psum_o += W2_g^T @ feat_g
All matmuls use float32r (fp32 data, full PE rate at N>=256, ~1e-4 rel err).
"""

import numpy as np

import concourse.bacc as bacc
import concourse.tile as tile
from concourse import mybir
from concourse.bass_utils import run_bass_kernel_spmd

F32 = mybir.dt.float32
F32R = mybir.dt.float32r

B, C, H, W = 8, 128, 64, 64
NUMS, HEADS, OUT = 16, 8, 256
P = H * W          # 4096 pixels per image
PB = 512           # pixel block (one PSUM bank of fp32)
NPB = P // PB      # 8 pixel blocks
N_CORES = 8

_CACHE = {}


def _build():
    nc = bacc.Bacc("TRN2", target_bir_lowering=False, debug=False,
                   num_devices=N_CORES)

    x_d = nc.dram_tensor("x", [C, P], F32, kind="ExternalInput")
    w1s_d = nc.dram_tensor("w1s", [C, NUMS], F32, kind="ExternalInput")
    sel_d = nc.dram_tensor("sel", [NUMS, NUMS * C], F32, kind="ExternalInput")
    w2t_d = nc.dram_tensor("w2t", [C, NUMS * OUT], F32, kind="ExternalInput")
    b1_d = nc.dram_tensor("b1c", [NUMS, 1], F32, kind="ExternalInput")
    b2_d = nc.dram_tensor("b2c", [C, 2], F32, kind="ExternalInput")
    out_d = nc.dram_tensor("out", [OUT, P], F32, kind="ExternalOutput")

    with tile.TileContext(nc) as tc:
        with (
            tc.tile_pool(name="const", bufs=1) as cpool,
            tc.tile_pool(name="fmsb", bufs=2) as fmsb,
            tc.tile_pool(name="feat", bufs=4) as featp,
            tc.tile_pool(name="osb", bufs=4) as osb,
            tc.tile_pool(name="psfm", bufs=1, space="PSUM") as psfm,
            tc.tile_pool(name="psrep", bufs=3, space="PSUM") as psrep,
            tc.tile_pool(name="psout", bufs=4, space="PSUM") as psout,
        ):
            # ---- load + round constants / input ----
            x_t = cpool.tile([C, P], F32)
            nc.sync.dma_start(x_t[:], x_d[:])
            x_r = cpool.tile([C, P], F32R)
            nc.vector.tensor_copy(x_r[:], x_t[:])

            w1s_t = cpool.tile([C, NUMS], F32)
            nc.sync.dma_start(w1s_t[:], w1s_d[:])
            w1s_r = cpool.tile([C, NUMS], F32R)
            nc.vector.tensor_copy(w1s_r[:], w1s_t[:])

            sel_t = cpool.tile([NUMS, NUMS * C], F32)
            nc.sync.dma_start(sel_t[:], sel_d[:])
            sel_r = cpool.tile([NUMS, NUMS * C], F32R)
            nc.vector.tensor_copy(sel_r[:], sel_t[:])

            w2t_t = cpool.tile([C, NUMS * OUT], F32)
            nc.sync.dma_start(w2t_t[:], w2t_d[:])
            w2t_r = cpool.tile([C, NUMS * OUT], F32R)
            nc.vector.tensor_copy(w2t_r[:], w2t_t[:])

            b1_t = cpool.tile([NUMS, 1], F32)
            nc.sync.dma_start(b1_t[:], b1_d[:])
            b2_t = cpool.tile([C, 2], F32)
            nc.sync.dma_start(b2_t[:], b2_d[:])

            relu = mybir.ActivationFunctionType.Relu
            copy = mybir.ActivationFunctionType.Identity

            for pb in range(NPB):
                px = slice(pb * PB, (pb + 1) * PB)

                # fm = relu(W1s^T @ x + b1)   [16, PB]
                ps_fm = psfm.tile([NUMS, PB], F32)
                nc.tensor.matmul(ps_fm[:], w1s_r[:], x_r[:, px],
                                 start=True, stop=True)
                fm = fmsb.tile([NUMS, PB], F32R)
                nc.scalar.activation(fm[:], ps_fm[:], relu, bias=b1_t[:])

                ps_o0 = psout.tile([C, PB], F32, tag="pso")
                ps_o1 = psout.tile([C, PB], F32, tag="pso")

                for g in range(NUMS):
                    # broadcast fm row g to 128 partitions
                    ps_rep = psrep.tile([C, PB], F32)
                    nc.tensor.matmul(ps_rep[:],
                                     sel_r[:, g * C:(g + 1) * C],
                                     fm[:], start=True, stop=True)
                    # feat_g = x * fm_rep_g
                    ft = featp.tile([C, PB], F32R, tag="ft")
                    nc.vector.tensor_tensor(ft[:], x_r[:, px], ps_rep[:],
                                            op=mybir.AluOpType.mult)
                    # accumulate both output-channel chunks
                    nc.tensor.matmul(ps_o0[:],
                                     w2t_r[:, (2 * g) * C:(2 * g + 1) * C],
                                     ft[:], start=(g == 0), stop=(g == NUMS - 1))
                    nc.tensor.matmul(ps_o1[:],
                                     w2t_r[:, (2 * g + 1) * C:(2 * g + 2) * C],
                                     ft[:], start=(g == 0), stop=(g == NUMS - 1))

                o0 = osb.tile([C, PB], F32, tag="osb")
                o1 = osb.tile([C, PB], F32, tag="osb")
                nc.scalar.activation(o0[:], ps_o0[:], copy, bias=b2_t[:, 0:1])
                nc.scalar.activation(o1[:], ps_o1[:], copy, bias=b2_t[:, 1:2])
                nc.sync.dma_start(out_d[0:C, px], o0[:])
                nc.sync.dma_start(out_d[C:OUT, px], o1[:])

    nc.compile()
    return nc


def _prep_params(W1, b1, W2, b2):
    # w1s[c, g] = W1[g, c - 8g] for 8g <= c < 8(g+1), else 0
    w1s = np.zeros((C, NUMS), dtype=np.float32)
    for g in range(NUMS):
        w1s[g * HEADS:(g + 1) * HEADS, g] = W1[g]
    # sel[:, g*128:(g+1)*128] = one-hot column block (row g all ones)
    sel = np.zeros((NUMS, NUMS * C), dtype=np.float32)
    for g in range(NUMS):
        sel[g, g * C:(g + 1) * C] = 1.0
    # w2t[k, (g*2+oc)*128 + m] = W2[oc*128 + m, g*128 + k]
    w2t = (
        W2.reshape(2, C, NUMS, C)        # [oc, m, g, k]
        .transpose(3, 2, 0, 1)           # [k, g, oc, m]
        .reshape(C, NUMS * OUT)
        .astype(np.float32, copy=True)
    )
    b1c = b1.reshape(NUMS, 1).astype(np.float32, copy=True)
    b2c = b2.reshape(2, C).T.astype(np.float32, copy=True)
    return w1s, sel, w2t, b1c, b2c


def kernel(x, W1, b1, W2, b2, _trace=False, _trace_kwargs=None):
    if "nc" not in _CACHE:
        _CACHE["nc"] = _build()
    nc = _CACHE["nc"]

    w1s, sel, w2t, b1c, b2c = _prep_params(W1, b1, W2, b2)
    xs = np.ascontiguousarray(x.reshape(B, C, P).astype(np.float32))
    in_maps = [
        {"x": xs[b_], "w1s": w1s, "sel": sel, "w2t": w2t,
         "b1c": b1c, "b2c": b2c}
        for b_ in range(N_CORES)
    ]
    kwargs = {}
    if _trace:
        kwargs["trace"] = True
        kwargs.update(_trace_kwargs or {})
    res = run_bass_kernel_spmd(nc, in_maps, core_ids=list(range(N_CORES)),
                               **kwargs)
    out = np.stack([res.results[b_]["out"] for b_ in range(N_CORES)])
    out = out.reshape(B, OUT, H, W)
    if _trace:
        _CACHE["last_result"] = res
    return out


# revision 4
# speedup vs baseline: 1.4523x; 1.4523x over previous
"""Trainium2 Bass kernel for nn_CrossChannelAttention.

Reference computation (per batch b, pixel p, with C=128 channels, NUMS=16
groups of HEADS=8 channels, OUT=256):
    fm[g,p]  = relu(sum_h W1[g,h] * x[8g+h, p] + b1[g])          # [16, P]
    feat[(g,d), p] = fm[g,p] * x[d,p]                            # [2048, P]
    out[o,p] = sum_c W2[o,c] * feat[c,p] + b2[o]                 # [256, P]

Strategy: data-parallel over batch B=8 across the 8 NeuronCores (one batch
image per core, params replicated).  Per core:
  - fm via one small matmul (W1 scattered into a [128,16] one-hot-ish lhsT)
  - fm row g broadcast to 128 partitions via a selection matmul
    (lhsT = one-hot column block of sel [16,128], rhs = fm [16,512])
  - feat_g = x * fm_rep_g elementwise (split across vector + gpsimd engines)
  - out accumulated over the 16 groups in PSUM:  psum_o += W2_g^T @ feat_g
All matmuls in bf16 (1 col/cycle on the PE; fp32 PSUM accumulation).
"""

import numpy as np
import ml_dtypes

import concourse.bacc as bacc
import concourse.tile as tile
from concourse import mybir
from concourse.bass_utils import run_bass_kernel_spmd

F32 = mybir.dt.float32
BF16 = mybir.dt.bfloat16

B, C, H, W = 8, 128, 64, 64
NUMS, HEADS, OUT = 16, 8, 256
P = H * W          # 4096 pixels per image
PB = 512           # pixel block (one PSUM bank of fp32)
NPB = P // PB      # 8 pixel blocks
N_CORES = 8
GPSIMD_GS = 5      # how many of the 16 feat-multiplies go to gpsimd

_CACHE = {}


def _build():
    nc = bacc.Bacc("TRN2", target_bir_lowering=False, debug=False,
                   num_devices=N_CORES)

    x_d = nc.dram_tensor("x", [C, P], F32, kind="ExternalInput")
    w1s_d = nc.dram_tensor("w1s", [C, NUMS], BF16, kind="ExternalInput")
    sel_d = nc.dram_tensor("sel", [NUMS, NUMS * C], BF16, kind="ExternalInput")
    w2t_d = nc.dram_tensor("w2t", [C, NUMS * OUT], BF16, kind="ExternalInput")
    b1_d = nc.dram_tensor("b1c", [NUMS, 1], F32, kind="ExternalInput")
    b2_d = nc.dram_tensor("b2c", [C, 2], F32, kind="ExternalInput")
    out_d = nc.dram_tensor("out", [OUT, P], F32, kind="ExternalOutput")

    with tile.TileContext(nc) as tc:
        with (
            tc.tile_pool(name="const", bufs=1) as cpool,
            tc.tile_pool(name="fmsb", bufs=2) as fmsb,
            tc.tile_pool(name="feat", bufs=6) as featp,
            tc.tile_pool(name="osb", bufs=4) as osb,
            tc.tile_pool(name="psfm", bufs=1, space="PSUM") as psfm,
            tc.tile_pool(name="psrep", bufs=3, space="PSUM") as psrep,
            tc.tile_pool(name="psout", bufs=4, space="PSUM") as psout,
        ):
            # ---- load constants / input, cast x to bf16 ----
            x_t = cpool.tile([C, P], F32)
            nc.sync.dma_start(x_t[:], x_d[:])
            x_b = cpool.tile([C, P], BF16)
            nc.vector.tensor_copy(x_b[:], x_t[:])

            w1s_t = cpool.tile([C, NUMS], BF16)
            nc.sync.dma_start(w1s_t[:], w1s_d[:])
            sel_t = cpool.tile([NUMS, NUMS * C], BF16)
            nc.sync.dma_start(sel_t[:], sel_d[:])
            w2t_t = cpool.tile([C, NUMS * OUT], BF16)
            nc.sync.dma_start(w2t_t[:], w2t_d[:])
            b1_t = cpool.tile([NUMS, 1], F32)
            nc.sync.dma_start(b1_t[:], b1_d[:])
            b2_t = cpool.tile([C, 2], F32)
            nc.sync.dma_start(b2_t[:], b2_d[:])

            relu = mybir.ActivationFunctionType.Relu
            ident = mybir.ActivationFunctionType.Identity

            for pb in range(NPB):
                px = slice(pb * PB, (pb + 1) * PB)

                # fm = relu(W1s^T @ x + b1)   [16, PB]
                ps_fm = psfm.tile([NUMS, PB], F32)
                nc.tensor.matmul(ps_fm[:], w1s_t[:], x_b[:, px],
                                 start=True, stop=True)
                fm = fmsb.tile([NUMS, PB], BF16)
                nc.scalar.activation(fm[:], ps_fm[:], relu, bias=b1_t[:])

                ps_o0 = psout.tile([C, PB], F32, tag="pso")
                ps_o1 = psout.tile([C, PB], F32, tag="pso")

                for g in range(NUMS):
                    # broadcast fm row g to 128 partitions
                    ps_rep = psrep.tile([C, PB], F32)
                    nc.tensor.matmul(ps_rep[:],
                                     sel_t[:, g * C:(g + 1) * C],
                                     fm[:], start=True, stop=True)
                    # feat_g = x * fm_rep_g
                    ft = featp.tile([C, PB], BF16, tag="ft")
                    if g < GPSIMD_GS:
                        # gpsimd cannot read PSUM: stage rep through SBUF
                        rep_sb = featp.tile([C, PB], BF16, tag="repsb")
                        nc.scalar.copy(rep_sb[:], ps_rep[:])
                        nc.gpsimd.tensor_tensor(ft[:], x_b[:, px], rep_sb[:],
                                                op=mybir.AluOpType.mult)
                    else:
                        nc.vector.tensor_tensor(ft[:], x_b[:, px], ps_rep[:],
                                                op=mybir.AluOpType.mult)
                    # accumulate both output-channel chunks
                    nc.tensor.matmul(ps_o0[:],
                                     w2t_t[:, (2 * g) * C:(2 * g + 1) * C],
                                     ft[:], start=(g == 0), stop=(g == NUMS - 1))
                    nc.tensor.matmul(ps_o1[:],
                                     w2t_t[:, (2 * g + 1) * C:(2 * g + 2) * C],
                                     ft[:], start=(g == 0), stop=(g == NUMS - 1))

                o0 = osb.tile([C, PB], F32, tag="osb")
                o1 = osb.tile([C, PB], F32, tag="osb")
                nc.scalar.activation(o0[:], ps_o0[:], ident, bias=b2_t[:, 0:1])
                nc.scalar.activation(o1[:], ps_o1[:], ident, bias=b2_t[:, 1:2])
                nc.sync.dma_start(out_d[0:C, px], o0[:])
                nc.sync.dma_start(out_d[C:OUT, px], o1[:])

    nc.compile()
    return nc


def _prep_params(W1, b1, W2, b2):
    bf = ml_dtypes.bfloat16
    # w1s[c, g] = W1[g, c - 8g] for 8g <= c < 8(g+1), else 0
    w1s = np.zeros((C, NUMS), dtype=bf)
    for g in range(NUMS):
        w1s[g * HEADS:(g + 1) * HEADS, g] = W1[g].astype(bf)
    # sel[:, g*128:(g+1)*128] = one-hot column block (row g all ones)
    sel = np.zeros((NUMS, NUMS * C), dtype=bf)
    for g in range(NUMS):
        sel[g, g * C:(g + 1) * C] = bf(1.0)
    # w2t[k, (g*2+oc)*128 + m] = W2[oc*128 + m, g*128 + k]
    w2t = (
        np.asarray(W2, dtype=np.float32)
        .reshape(2, C, NUMS, C)          # [oc, m, g, k]
        .transpose(3, 2, 0, 1)           # [k, g, oc, m]
        .reshape(C, NUMS * OUT)
        .astype(bf)
    )
    b1c = np.asarray(b1, dtype=np.float32).reshape(NUMS, 1).copy()
    b2c = np.asarray(b2, dtype=np.float32).reshape(2, C).T.copy()
    return w1s, sel, w2t, b1c, b2c


def kernel(x, W1, b1, W2, b2, _trace=False, _trace_kwargs=None):
    if "nc" not in _CACHE:
        _CACHE["nc"] = _build()
    nc = _CACHE["nc"]

    w1s, sel, w2t, b1c, b2c = _prep_params(W1, b1, W2, b2)
    xs = np.ascontiguousarray(np.asarray(x, dtype=np.float32).reshape(B, C, P))
    in_maps = [
        {"x": xs[b_], "w1s": w1s, "sel": sel, "w2t": w2t,
         "b1c": b1c, "b2c": b2c}
        for b_ in range(N_CORES)
    ]
    kwargs = {}
    if _trace:
        kwargs["trace"] = True
        kwargs.update(_trace_kwargs or {})
    res = run_bass_kernel_spmd(nc, in_maps, core_ids=list(range(N_CORES)),
                               **kwargs)
    out = np.stack([res.results[b_]["out"] for b_ in range(N_CORES)])
    out = out.reshape(B, OUT, H, W)
    if _trace:
        _CACHE["last_result"] = res
    return out


# revision 5
# speedup vs baseline: 1.6205x; 1.1158x over previous
"""Trainium2 Bass kernel for nn_CrossChannelAttention.

Reference computation (per batch b, pixel p, with C=128 channels, NUMS=16
groups of HEADS=8 channels, OUT=256):
    fm[g,p]  = relu(sum_h W1[g,h] * x[8g+h, p] + b1[g])          # [16, P]
    feat[(g,d), p] = fm[g,p] * x[d,p]                            # [2048, P]
    out[o,p] = sum_c W2[o,c] * feat[c,p] + b2[o]                 # [256, P]

Strategy: data-parallel over batch B=8 across the 8 NeuronCores (one batch
image per core, params replicated).  Per core:
  - prologue: all fm tiles via small matmuls (W1 scattered into [128,16])
  - per pixel-block, software-pipelined over the 16 groups with the
    replication matmuls running LOOKAHEAD iterations ahead of the main
    accumulating matmuls, so the PE never stalls on the vector engine:
      rep:  fm row g -> 128 partitions (selection matmul, PSUM)
      feat: x * fm_rep on vector engine (or gpsimd via an ACT-staged copy)
      main: psum_o{0,1} += W2_g^T @ feat_g
All matmuls in bf16 (fp32 PSUM accumulation); rel err ~4e-3.
"""

import numpy as np
import ml_dtypes

import concourse.bacc as bacc
import concourse.tile as tile
from concourse import mybir
from concourse.bass_utils import run_bass_kernel_spmd

F32 = mybir.dt.float32
BF16 = mybir.dt.bfloat16

B, C, H, W = 8, 128, 64, 64
NUMS, HEADS, OUT = 16, 8, 256
P = H * W          # 4096 pixels per image
PB = 512           # pixel block (one PSUM bank of fp32)
NPB = P // PB      # 8 pixel blocks
N_CORES = 8
LOOKAHEAD = 3      # rep/feat pipeline depth ahead of main matmuls
GPSIMD_GS = {12, 13, 14, 15}   # groups whose feat-multiply runs on gpsimd

_CACHE = {}


def _build():
    nc = bacc.Bacc("TRN2", target_bir_lowering=False, debug=False,
                   num_devices=N_CORES)

    x_d = nc.dram_tensor("x", [C, P], F32, kind="ExternalInput")
    w1s_d = nc.dram_tensor("w1s", [C, NUMS], BF16, kind="ExternalInput")
    sel_d = nc.dram_tensor("sel", [NUMS, NUMS * C], BF16, kind="ExternalInput")
    w2t_d = nc.dram_tensor("w2t", [C, NUMS * OUT], BF16, kind="ExternalInput")
    b1_d = nc.dram_tensor("b1c", [NUMS, 1], F32, kind="ExternalInput")
    b2_d = nc.dram_tensor("b2c", [C, 2], F32, kind="ExternalInput")
    out_d = nc.dram_tensor("out", [OUT, P], F32, kind="ExternalOutput")

    relu = mybir.ActivationFunctionType.Relu
    ident = mybir.ActivationFunctionType.Identity
    mult = mybir.AluOpType.mult

    with tile.TileContext(nc) as tc:
        with (
            tc.tile_pool(name="const", bufs=1) as cpool,
            tc.tile_pool(name="xp", bufs=1) as xp,
            tc.tile_pool(name="feat", bufs=2 * (LOOKAHEAD + 2)) as featp,
            tc.tile_pool(name="osb", bufs=4) as osb,
            tc.tile_pool(name="psfm", bufs=2, space="PSUM") as psfm,
            tc.tile_pool(name="psrep", bufs=4, space="PSUM") as psrep,
            tc.tile_pool(name="psout", bufs=2, space="PSUM") as psout,
        ):
            # ---- constants ----
            w1s_t = cpool.tile([C, NUMS], BF16)
            nc.sync.dma_start(w1s_t[:], w1s_d[:])
            sel_t = cpool.tile([NUMS, NUMS * C], BF16)
            nc.sync.dma_start(sel_t[:], sel_d[:])
            w2t_t = cpool.tile([C, NUMS * OUT], BF16)
            nc.sync.dma_start(w2t_t[:], w2t_d[:])
            b1_t = cpool.tile([NUMS, 1], F32)
            nc.sync.dma_start(b1_t[:], b1_d[:])
            b2_t = cpool.tile([C, 2], F32)
            nc.sync.dma_start(b2_t[:], b2_d[:])

            # ---- x: load per pixel-block, cast to bf16 ----
            xbs = []
            for pb in range(NPB):
                px = slice(pb * PB, (pb + 1) * PB)
                x_t = xp.tile([C, PB], F32, tag=f"x{pb}")
                nc.sync.dma_start(x_t[:], x_d[:, px])
                x_b = xp.tile([C, PB], BF16, tag=f"xb{pb}")
                nc.vector.tensor_copy(x_b[:], x_t[:])
                xbs.append(x_b)

            # ---- prologue: fm for all pixel blocks ----
            fm_sb = cpool.tile([NUMS, P], BF16)
            for pb in range(NPB):
                px = slice(pb * PB, (pb + 1) * PB)
                ps_fm = psfm.tile([NUMS, PB], F32)
                nc.tensor.matmul(ps_fm[:], w1s_t[:], xbs[pb][:],
                                 start=True, stop=True)
                nc.scalar.activation(fm_sb[:, px], ps_fm[:], relu, bias=b1_t[:])

            # ---- main: software-pipelined over groups ----
            for pb in range(NPB):
                px = slice(pb * PB, (pb + 1) * PB)
                ps_o0 = psout.tile([C, PB], F32, tag="pso")
                ps_o1 = psout.tile([C, PB], F32, tag="pso")
                fts = {}

                def emit_rep(g, pb=pb, px=px, fts=fts):
                    ps_rep = psrep.tile([C, PB], F32, tag="rep")
                    nc.tensor.matmul(ps_rep[:],
                                     sel_t[:, g * C:(g + 1) * C],
                                     fm_sb[:, px], start=True, stop=True)
                    ft = featp.tile([C, PB], BF16, tag="ft")
                    if g in GPSIMD_GS:
                        rep_sb = featp.tile([C, PB], BF16, tag="repsb")
                        nc.scalar.copy(rep_sb[:], ps_rep[:])
                        nc.gpsimd.tensor_tensor(ft[:], xbs[pb][:], rep_sb[:],
                                                op=mult)
                    else:
                        nc.vector.tensor_tensor(ft[:], xbs[pb][:], ps_rep[:],
                                                op=mult)
                    fts[g] = ft

                for g in range(LOOKAHEAD):
                    emit_rep(g)
                for g in range(NUMS):
                    if g + LOOKAHEAD < NUMS:
                        emit_rep(g + LOOKAHEAD)
                    ft = fts.pop(g)
                    nc.tensor.matmul(ps_o0[:],
                                     w2t_t[:, (2 * g) * C:(2 * g + 1) * C],
                                     ft[:], start=(g == 0), stop=(g == NUMS - 1))
                    nc.tensor.matmul(ps_o1[:],
                                     w2t_t[:, (2 * g + 1) * C:(2 * g + 2) * C],
                                     ft[:], start=(g == 0), stop=(g == NUMS - 1))

                o0 = osb.tile([C, PB], F32, tag="osb")
                o1 = osb.tile([C, PB], F32, tag="osb")
                nc.scalar.activation(o0[:], ps_o0[:], ident, bias=b2_t[:, 0:1])
                nc.scalar.activation(o1[:], ps_o1[:], ident, bias=b2_t[:, 1:2])
                nc.sync.dma_start(out_d[0:C, px], o0[:])
                nc.sync.dma_start(out_d[C:OUT, px], o1[:])

    nc.compile()
    return nc


def _prep_params(W1, b1, W2, b2):
    bf = ml_dtypes.bfloat16
    # w1s[c, g] = W1[g, c - 8g] for 8g <= c < 8(g+1), else 0
    w1s = np.zeros((C, NUMS), dtype=bf)
    for g in range(NUMS):
        w1s[g * HEADS:(g + 1) * HEADS, g] = W1[g].astype(bf)
    # sel[:, g*128:(g+1)*128] = one-hot column block (row g all ones)
    sel = np.zeros((NUMS, NUMS * C), dtype=bf)
    for g in range(NUMS):
        sel[g, g * C:(g + 1) * C] = bf(1.0)
    # w2t[k, (g*2+oc)*128 + m] = W2[oc*128 + m, g*128 + k]
    w2t = (
        np.asarray(W2, dtype=np.float32)
        .reshape(2, C, NUMS, C)          # [oc, m, g, k]
        .transpose(3, 2, 0, 1)           # [k, g, oc, m]
        .reshape(C, NUMS * OUT)
        .astype(bf)
    )
    b1c = np.asarray(b1, dtype=np.float32).reshape(NUMS, 1).copy()
    b2c = np.asarray(b2, dtype=np.float32).reshape(2, C).T.copy()
    return w1s, sel, w2t, b1c, b2c


def kernel(x, W1, b1, W2, b2, _trace=False, _trace_kwargs=None):
    if "nc" not in _CACHE:
        _CACHE["nc"] = _build()
    nc = _CACHE["nc"]

    w1s, sel, w2t, b1c, b2c = _prep_params(W1, b1, W2, b2)
    xs = np.ascontiguousarray(np.asarray(x, dtype=np.float32).reshape(B, C, P))
    in_maps = [
        {"x": xs[b_], "w1s": w1s, "sel": sel, "w2t": w2t,
         "b1c": b1c, "b2c": b2c}
        for b_ in range(N_CORES)
    ]
    kwargs = {}
    if _trace:
        kwargs["trace"] = True
        kwargs.update(_trace_kwargs or {})
    res = run_bass_kernel_spmd(nc, in_maps, core_ids=list(range(N_CORES)),
                               **kwargs)
    out = np.stack([res.results[b_]["out"] for b_ in range(N_CORES)])
    out = out.reshape(B, OUT, H, W)
    if _trace:
        _CACHE["last_result"] = res
    return out


# revision 6
# speedup vs baseline: 1.6634x; 1.0264x over previous
"""Trainium2 Bass kernel for nn_CrossChannelAttention.

Reference computation (per batch b, pixel p, with C=128 channels, NUMS=16
groups of HEADS=8 channels, OUT=256):
    fm[g,p]  = relu(sum_h W1[g,h] * x[8g+h, p] + b1[g])          # [16, P]
    feat[(g,d), p] = fm[g,p] * x[d,p]                            # [2048, P]
    out[o,p] = sum_c W2[o,c] * feat[c,p] + b2[o]                 # [256, P]

Strategy: data-parallel over batch B=8 across the 8 NeuronCores (one batch
image per core, params replicated).  Per core:
  - prologue: all fm tiles via small matmuls (W1 scattered into [128,16])
  - per pixel-block, software-pipelined over the 16 groups with the
    replication matmuls running LOOKAHEAD iterations ahead of the main
    accumulating matmuls, so the PE never stalls on the vector engine:
      rep:  fm row g -> 128 partitions (selection matmul, PSUM)
      feat: x * fm_rep on vector engine (or gpsimd via an ACT-staged copy)
      main: psum_o{0,1} += W2_g^T @ feat_g
All matmuls in bf16 (fp32 PSUM accumulation); rel err ~4e-3.
"""

import numpy as np
import ml_dtypes

import concourse.bacc as bacc
import concourse.tile as tile
from concourse import mybir
from concourse.bass_utils import run_bass_kernel_spmd

F32 = mybir.dt.float32
BF16 = mybir.dt.bfloat16

B, C, H, W = 8, 128, 64, 64
NUMS, HEADS, OUT = 16, 8, 256
P = H * W          # 4096 pixels per image
PB = 512           # pixel block (one PSUM bank of fp32)
NPB = P // PB      # 8 pixel blocks
N_CORES = 8
LOOKAHEAD = 5      # rep/feat pipeline depth ahead of main matmuls
GPSIMD_GS = {13, 14, 15}   # groups whose feat-multiply runs on gpsimd

_CACHE = {}


def _build():
    nc = bacc.Bacc("TRN2", target_bir_lowering=False, debug=False,
                   num_devices=N_CORES)

    x_d = nc.dram_tensor("x", [C, P], F32, kind="ExternalInput")
    w1s_d = nc.dram_tensor("w1s", [C, NUMS], BF16, kind="ExternalInput")
    sel_d = nc.dram_tensor("sel", [NUMS, NUMS * C], BF16, kind="ExternalInput")
    w2t_d = nc.dram_tensor("w2t", [C, NUMS * OUT], BF16, kind="ExternalInput")
    b1_d = nc.dram_tensor("b1c", [NUMS, 1], F32, kind="ExternalInput")
    b2_d = nc.dram_tensor("b2c", [C, 2], F32, kind="ExternalInput")
    out_d = nc.dram_tensor("out", [OUT, P], F32, kind="ExternalOutput")

    relu = mybir.ActivationFunctionType.Relu
    ident = mybir.ActivationFunctionType.Identity
    mult = mybir.AluOpType.mult

    with tile.TileContext(nc) as tc:
        with (
            tc.tile_pool(name="const", bufs=1) as cpool,
            tc.tile_pool(name="xp", bufs=1) as xp,
            tc.tile_pool(name="feat", bufs=2 * (LOOKAHEAD + 2)) as featp,
            tc.tile_pool(name="osb", bufs=4) as osb,
            tc.tile_pool(name="psrep", bufs=6, space="PSUM") as psrep,
            tc.tile_pool(name="psout", bufs=2, space="PSUM") as psout,
        ):
            # ---- x: load per pixel-block, cast to bf16 (gpsimd) ----
            xbs = []
            for pb in range(NPB):
                px = slice(pb * PB, (pb + 1) * PB)
                x_t = xp.tile([C, PB], F32, tag=f"x{pb}")
                nc.sync.dma_start(x_t[:], x_d[:, px])
                x_b = xp.tile([C, PB], BF16, tag=f"xb{pb}")
                nc.gpsimd.tensor_copy(x_b[:], x_t[:])
                xbs.append(x_b)

            # ---- constants ----
            w1s_t = cpool.tile([C, NUMS], BF16)
            nc.sync.dma_start(w1s_t[:], w1s_d[:])
            sel_t = cpool.tile([NUMS, NUMS * C], BF16)
            nc.sync.dma_start(sel_t[:], sel_d[:])
            w2t_t = cpool.tile([C, NUMS * OUT], BF16)
            nc.sync.dma_start(w2t_t[:], w2t_d[:])
            b1_t = cpool.tile([NUMS, 1], F32)
            nc.sync.dma_start(b1_t[:], b1_d[:])
            b2_t = cpool.tile([C, 2], F32)
            nc.sync.dma_start(b2_t[:], b2_d[:])

            # ---- prologue: fm for all pixel blocks ----
            fm_sb = cpool.tile([NUMS, P], BF16)
            for pb in range(NPB):
                px = slice(pb * PB, (pb + 1) * PB)
                ps_fm = psrep.tile([NUMS, PB], F32, tag="rep")
                nc.tensor.matmul(ps_fm[:], w1s_t[:], xbs[pb][:],
                                 start=True, stop=True)
                nc.scalar.activation(fm_sb[:, px], ps_fm[:], relu, bias=b1_t[:])

            # ---- main: software-pipelined over groups ----
            for pb in range(NPB):
                px = slice(pb * PB, (pb + 1) * PB)
                ps_o0 = psout.tile([C, PB], F32, tag="pso")
                ps_o1 = psout.tile([C, PB], F32, tag="pso")
                fts = {}

                def emit_rep(g, pb=pb, px=px, fts=fts):
                    ps_rep = psrep.tile([C, PB], F32, tag="rep")
                    nc.tensor.matmul(ps_rep[:],
                                     sel_t[:, g * C:(g + 1) * C],
                                     fm_sb[:, px], start=True, stop=True)
                    ft = featp.tile([C, PB], BF16, tag="ft")
                    if g in GPSIMD_GS:
                        rep_sb = featp.tile([C, PB], BF16, tag="repsb")
                        nc.scalar.copy(rep_sb[:], ps_rep[:])
                        nc.gpsimd.tensor_tensor(ft[:], xbs[pb][:], rep_sb[:],
                                                op=mult)
                    else:
                        nc.vector.tensor_tensor(ft[:], xbs[pb][:], ps_rep[:],
                                                op=mult)
                    fts[g] = ft

                for g in range(LOOKAHEAD):
                    emit_rep(g)
                for g in range(NUMS):
                    if g + LOOKAHEAD < NUMS:
                        emit_rep(g + LOOKAHEAD)
                    ft = fts.pop(g)
                    nc.tensor.matmul(ps_o0[:],
                                     w2t_t[:, (2 * g) * C:(2 * g + 1) * C],
                                     ft[:], start=(g == 0), stop=(g == NUMS - 1))
                    nc.tensor.matmul(ps_o1[:],
                                     w2t_t[:, (2 * g + 1) * C:(2 * g + 2) * C],
                                     ft[:], start=(g == 0), stop=(g == NUMS - 1))

                o0 = osb.tile([C, PB], F32, tag="osb")
                o1 = osb.tile([C, PB], F32, tag="osb")
                nc.scalar.activation(o0[:], ps_o0[:], ident, bias=b2_t[:, 0:1])
                nc.scalar.activation(o1[:], ps_o1[:], ident, bias=b2_t[:, 1:2])
                nc.sync.dma_start(out_d[0:C, px], o0[:])
                nc.sync.dma_start(out_d[C:OUT, px], o1[:])

    nc.compile()
    return nc


def _prep_params(W1, b1, W2, b2):
    bf = ml_dtypes.bfloat16
    # w1s[c, g] = W1[g, c - 8g] for 8g <= c < 8(g+1), else 0
    w1s = np.zeros((C, NUMS), dtype=bf)
    for g in range(NUMS):
        w1s[g * HEADS:(g + 1) * HEADS, g] = W1[g].astype(bf)
    # sel[:, g*128:(g+1)*128] = one-hot column block (row g all ones)
    sel = np.zeros((NUMS, NUMS * C), dtype=bf)
    for g in range(NUMS):
        sel[g, g * C:(g + 1) * C] = bf(1.0)
    # w2t[k, (g*2+oc)*128 + m] = W2[oc*128 + m, g*128 + k]
    w2t = (
        np.asarray(W2, dtype=np.float32)
        .reshape(2, C, NUMS, C)          # [oc, m, g, k]
        .transpose(3, 2, 0, 1)           # [k, g, oc, m]
        .reshape(C, NUMS * OUT)
        .astype(bf)
    )
    b1c = np.asarray(b1, dtype=np.float32).reshape(NUMS, 1).copy()
    b2c = np.asarray(b2, dtype=np.float32).reshape(2, C).T.copy()
    return w1s, sel, w2t, b1c, b2c


def kernel(x, W1, b1, W2, b2, _trace=False, _trace_kwargs=None):
    if "nc" not in _CACHE:
        _CACHE["nc"] = _build()
    nc = _CACHE["nc"]

    w1s, sel, w2t, b1c, b2c = _prep_params(W1, b1, W2, b2)
    xs = np.ascontiguousarray(np.asarray(x, dtype=np.float32).reshape(B, C, P))
    in_maps = [
        {"x": xs[b_], "w1s": w1s, "sel": sel, "w2t": w2t,
         "b1c": b1c, "b2c": b2c}
        for b_ in range(N_CORES)
    ]
    kwargs = {}
    if _trace:
        kwargs["trace"] = True
        kwargs.update(_trace_kwargs or {})
    res = run_bass_kernel_spmd(nc, in_maps, core_ids=list(range(N_CORES)),
                               **kwargs)
    out = np.stack([res.results[b_]["out"] for b_ in range(N_CORES)])
    out = out.reshape(B, OUT, H, W)
    if _trace:
        _CACHE["last_result"] = res
    return out
